# revision 26
# baseline (speedup 1.0000x reference)
"""MPNN layer on 8 Trainium2 NeuronCores (Bass/Tile).

Math (reference):
    m_edge = relu(x[dst] @ W1a^T + x[src] @ W1b^T + h @ W1c^T)        [E, D]
    m_node = segment_sum(m_edge, dst, N)                               [N, D]
    y      = m_node @ W2^T                                             [N, D]
    out_e  = relu(LN(snorm_e * y[src_e]))                              [E, D]
LN decomposition (exact):
    LN(s*v) = (v - mu_v) * s * rsqrt(s^2 * var_v + eps) * gamma + beta
so per-node stats (mu, var) are computed once per node and the per-edge part
is a scalar a_e = s_e * rsqrt(s_e^2 * var + eps) applied to the centered,
gamma-scaled node vector.

Sharding: edges partitioned by dst-bucket (node range) for phase 1 (each core
owns the complete segment-sum for its 1/8 of nodes - no reduction collective),
records (centered y + var) AllGathered, then phase 2 processes edges in
original order 1/8 chunks.

Segment-sum on PE: edges sorted by dst within a core; per 128-node block the
edge tiles matmul-accumulate (lhsT=m_edge tile [edge,feat], rhs=one-hot
[edge,node_rel]) into a psum [feat, node_rel]. One-hot built on DVE by
is_equal(iota_row, dst_rel); padded edge slots carry dst_rel=-1 giving a zero
one-hot column (exact zero contribution).

dma_gather uses int16 indices (<32768), so gathers from >32768-row tables are
split into a low call (rows [0,32768)) and a high call (rows [32768,...) with
indices rebased by -32768); edge slots are grouped [lo | hi] per block
(phase 1) / per shard (phase 2) so each call sees one range.
"""

import numpy as np
import ml_dtypes

from concourse import bacc, tile, mybir
from concourse import library_config
from concourse.bass_utils import run_bass_kernel_spmd

P = 128
LN_EPS = 1e-5
REC_W = 192            # record row: [yc(128) | var | pad..] f32; 768B (256B-mult)
BF16 = ml_dtypes.bfloat16

# ----------------------------------------------------------------------------
# host-side preprocessing
# ----------------------------------------------------------------------------


def _ceil_to(x, m):
    return -(-x // m) * m


def _wrap16(idx, dtype=np.int16):
    """[n] -> [128, n//16] int16: idx i at partition i%16, col i//16, replicated
    over the 8 groups of 16 partitions (each gpsimd q7 core reads its own 16)."""
    n = idx.shape[0]
    assert n % 16 == 0
    w = idx.reshape(n // 16, 16).T.astype(dtype)  # [16, n//16]
    return np.tile(w, (8, 1))


def _wrap128_cols(vals, n_tiles, fill, dtype=np.float32):
    """[n] -> [128, n_tiles]: value i at [i%128, i//128]; padded with fill."""
    out = np.full((n_tiles * P,), fill, dtype=dtype)
    out[: vals.shape[0]] = vals
    return out.reshape(n_tiles, P).T.copy()


class Plan:
    """All data-dependent layout decisions, computed on host from the inputs."""

    def __init__(self, n_nodes, n_edges, src, dst, nc=8, lo_limit=32768,
                 blk_nodes=128, p2_chunk_tiles=16):
        self.nc = nc
        self.n_nodes = n_nodes
        self.n_edges = n_edges
        self.lo_limit = lo_limit
        self.npc = n_nodes // nc                       # real nodes per core
        assert self.npc * nc == n_nodes
        self.npc_pad = _ceil_to(self.npc, blk_nodes)
        self.nblk = self.npc_pad // blk_nodes
        self.n_pad = self.npc_pad * nc                 # padded node table rows
        self.epc = n_edges // nc                       # phase-2 edges per core
        assert self.epc * nc == n_edges
        self.p2_chunk_tiles = p2_chunk_tiles

        src = np.asarray(src).astype(np.int64)
        dst = np.asarray(dst).astype(np.int64)
        self.src, self.dst = src, dst

        # ---- phase 1: bucket edges by dst core / block, lo/hi by src
        core_of = dst // self.npc
        blk_of = (dst - core_of * self.npc) // blk_nodes
        # mapped index of a node in slice-padded node tables (xb, records)
        self.node_map = (np.arange(n_nodes) // self.npc) * self.npc_pad + \
            (np.arange(n_nodes) % self.npc)
        is_lo1 = self.node_map[src] < lo_limit
        self.p1 = []       # per core: dict with per-block lo/hi edge id lists
        tl, th = 1, 0
        for c in range(nc):
            blocks = []
            in_c = core_of == c
            for b in range(self.nblk):
                m = in_c & (blk_of == b)
                lo_ids = np.nonzero(m & is_lo1)[0]
                hi_ids = np.nonzero(m & ~is_lo1)[0]
                blocks.append((lo_ids, hi_ids))
                tl = max(tl, -(-len(lo_ids) // P))
                th = max(th, -(-len(hi_ids) // P))
            self.p1.append(blocks)
        self.tl, self.th = tl, th
        self.t_blk = tl + th                            # tiles per block
        self.t1 = self.nblk * self.t_blk                # phase-1 tiles per core
        self.e1 = self.t1 * P

        # ---- phase 2: original-order shards, lo/hi by mapped src
        self.mapped_src = self.node_map[src]
        lo2_max, hi2_max = 1, 0
        self.p2 = []
        for c in range(nc):
            ids = np.arange(c * self.epc, (c + 1) * self.epc)
            m = self.mapped_src[ids] < lo_limit
            lo_ids, hi_ids = ids[m], ids[~m]
            self.p2.append((lo_ids, hi_ids))
            lo2_max = max(lo2_max, len(lo_ids))
            hi2_max = max(hi2_max, len(hi_ids))
        ct = p2_chunk_tiles * P
        self.lo2 = _ceil_to(lo2_max, ct) // P           # tiles in lo region
        self.hi2 = _ceil_to(hi2_max, ct) // P
        self.t2 = self.lo2 + self.hi2
        self.e2 = self.t2 * P

    # ---- per-core input arrays -------------------------------------------
    def core_inputs(self, c, x, h, snorm_n, W1, W2):
        p = self
        f32, i16 = np.float32, np.int16

        # phase-1 slot -> edge id (-1 for pad)
        slots = np.full(p.e1, -1, dtype=np.int64)
        for b, (lo_ids, hi_ids) in enumerate(p.p1[c]):
            base = b * p.t_blk * P
            slots[base: base + len(lo_ids)] = lo_ids
            base += p.tl * P
            slots[base: base + len(hi_ids)] = hi_ids
        pad = slots < 0
        e_ids = np.where(pad, 0, slots)

        h_t = np.ascontiguousarray(h[e_ids].T).astype(BF16)
        h_t[:, pad] = BF16(0.0)

        dst_loc = self.dst[e_ids] - c * p.npc
        dst_rel = dst_loc - (np.arange(p.e1) // (p.t_blk * P)) * 128
        dst_rel = np.where(pad, -1.0, dst_rel.astype(f32))
        dst_rel_w = dst_rel.reshape(p.t1, P).T.copy().astype(f32)  # [128, t1]

        idx_xa = np.where(pad, 0, dst_loc).astype(np.int64)
        src1 = np.where(pad, 0, self.node_map[self.src[e_ids]])
        # hi slots: rebase by lo_limit (pads in hi region -> 0)
        in_hi = np.zeros(p.e1, dtype=bool)
        for b in range(p.nblk):
            s = b * p.t_blk * P + p.tl * P
            in_hi[s: s + p.th * P] = True
        idx_xb = np.where(in_hi, np.maximum(src1 - p.lo_limit, 0), src1)
        idx_xb = np.where(pad, 0, idx_xb)

        # phase 2
        lo_ids, hi_ids = p.p2[c]
        slots2 = np.full(p.e2, -1, dtype=np.int64)
        slots2[: len(lo_ids)] = lo_ids
        slots2[p.lo2 * P: p.lo2 * P + len(hi_ids)] = hi_ids
        pad2 = slots2 < 0
        e2_ids = np.where(pad2, 0, slots2)
        mapped = self.mapped_src[e2_ids]
        idx_rec = np.where(np.arange(p.e2) >= p.lo2 * P,
                           np.maximum(mapped - p.lo_limit, 0), mapped)
        idx_rec = np.where(pad2, 0, idx_rec)
        sn = snorm_n.reshape(-1)[e2_ids].astype(f32)
        sn = np.where(pad2, 1.0, sn)

        return {
            "h_t": h_t,
            "dst_rel": dst_rel_w,
            "idx_xb": _wrap16(idx_xb),
            "idx_rec": _wrap16(idx_rec),
            "snorm": _wrap128_cols(sn, p.t2, 1.0),
        }, slots2


# ----------------------------------------------------------------------------
# bass program
# ----------------------------------------------------------------------------


def build_program(p: Plan, use_gamma: bool, use_beta: bool, stage="full"):
    # stage in {tables, phase1, ag, full} - debug bisect: later stages omitted

    dt = mybir.dt
    nc = bacc.Bacc(None)
    nc.gpsimd.load_library(library_config.mlp)

    n_xt = p.n_pad                  # node table rows (x_t cols)
    lo_rows = min(p.lo_limit, n_xt)
    hi_rows = n_xt - lo_rows

    # ---- parameters (per-core shapes; replicated arrays passed identically)
    x_t = nc.declare_dram_parameter("x_t", [P, n_xt], dt.bfloat16, isOutput=False)
    x_tl = nc.declare_dram_parameter("x_tl", [P, p.npc_pad], dt.bfloat16, isOutput=False)
    h_t = nc.declare_dram_parameter("h_t", [P, p.e1], dt.bfloat16, isOutput=False)
    w1aT = nc.declare_dram_parameter("w1aT", [P, P], dt.bfloat16, isOutput=False)
    w1bT = nc.declare_dram_parameter("w1bT", [P, P], dt.bfloat16, isOutput=False)
    w1cT = nc.declare_dram_parameter("w1cT", [P, P], dt.bfloat16, isOutput=False)
    w2T = nc.declare_dram_parameter("w2T", [P, P], dt.bfloat16, isOutput=False)
    ident_in = nc.declare_dram_parameter("ident", [P, P], dt.bfloat16, isOutput=False)
    iota_in = nc.declare_dram_parameter("iota", [P, P], dt.float32, isOutput=False)
    dst_rel = nc.declare_dram_parameter("dst_rel", [P, p.t1], dt.float32, isOutput=False)
    idx_xb = nc.declare_dram_parameter("idx_xb", [P, p.e1 // 16], dt.int16, isOutput=False)
    idx_rec = nc.declare_dram_parameter("idx_rec", [P, p.e2 // 16], dt.int16, isOutput=False)
    snorm = nc.declare_dram_parameter("snorm", [P, p.t2], dt.float32, isOutput=False)
    gamma_b = beta_b = None
    if use_gamma:
        gamma_b = nc.declare_dram_parameter("gamma_b", [P, P], dt.float32, isOutput=False)
    if use_beta:
        beta_b = nc.declare_dram_parameter("beta_b", [P, P], dt.float32, isOutput=False)

    out = nc.declare_dram_parameter("out", [p.e2, P], dt.float32, isOutput=True)

    # ---- internal DRAM
    xb_dram = nc.dram_tensor("xb_dram", [n_xt, P], dt.float32)
    rec_local = nc.dram_tensor("rec_local", [p.npc_pad, REC_W], dt.float32)
    rec_addr_space = "Shared" if p.nc > 4 else "Local"
    rec_full = nc.dram_tensor("rec_full", [p.n_pad, REC_W], dt.float32,
                              addr_space=rec_addr_space)

    f32, bf16 = dt.float32, dt.bfloat16
    GMAX = 8    # dma_gather is limited to 1024 indices (8 tiles) per call

    def gather_tiles(out_tile, in_ap, idx_sb, slot0, n_tiles, elem, tile_off=0):
        for g0 in range(0, n_tiles, GMAX):
            gn = min(GMAX, n_tiles - g0)
            e0 = slot0 + g0 * P
            nc.gpsimd.dma_gather(
                out_ap=out_tile[:, tile_off + g0: tile_off + g0 + gn, :],
                in_ap=in_ap,
                idxs_ap=idx_sb[:, e0 // 16: (e0 + gn * P) // 16],
                num_idxs=gn * P, num_idxs_reg=gn * P, elem_size=elem)

    with tile.TileContext(nc) as tc:
        with tc.tile_pool(name="const", bufs=1) as cpool, \
             tc.tile_pool(name="xtile", bufs=3) as xpool, \
             tc.tile_pool(name="tabout", bufs=3) as tpool, \
             tc.tile_pool(name="blk", bufs=2) as bpool, \
             tc.tile_pool(name="edge", bufs=3) as epool, \
             tc.tile_pool(name="nodeep", bufs=2) as npool, \
             tc.tile_pool(name="p2", bufs=2) as p2pool, \
             tc.tile_pool(name="psA", bufs=2, space="PSUM") as psA, \
             tc.tile_pool(name="psT", bufs=2, space="PSUM") as psT, \
             tc.tile_pool(name="psSeg", bufs=2, space="PSUM") as psSeg, \
             tc.tile_pool(name="psY", bufs=2, space="PSUM") as psY:

            # ---- constants
            w1aT_sb = cpool.tile([P, P], bf16, tag="w1a")
            w1bT_sb = cpool.tile([P, P], bf16, tag="w1b")
            w1cT_sb = cpool.tile([P, P], bf16, tag="w1c")
            w2T_sb = cpool.tile([P, P], bf16, tag="w2")
            ident_sb = cpool.tile([P, P], bf16, tag="ident")
            iota_sb = cpool.tile([P, P], f32, tag="iota")
            dstrel_sb = cpool.tile([P, p.t1], f32, tag="dstrel")
            ixb_sb = cpool.tile([P, p.e1 // 16], dt.int16, tag="ixb")
            irec_sb = cpool.tile([P, p.e2 // 16], dt.int16, tag="irec")
            snorm_sb = cpool.tile([P, p.t2], f32, tag="snorm")
            eps_sb = cpool.tile([P, 1], f32, tag="eps")
            nc.vector.memset(eps_sb[:], LN_EPS)
            nc.sync.dma_start(out=w1aT_sb[:], in_=w1aT[:])
            nc.sync.dma_start(out=w1bT_sb[:], in_=w1bT[:])
            nc.sync.dma_start(out=w1cT_sb[:], in_=w1cT[:])
            nc.sync.dma_start(out=w2T_sb[:], in_=w2T[:])
            nc.sync.dma_start(out=ident_sb[:], in_=ident_in[:])
            nc.sync.dma_start(out=iota_sb[:], in_=iota_in[:])
            nc.sync.dma_start(out=dstrel_sb[:], in_=dst_rel[:])
            nc.sync.dma_start(out=ixb_sb[:], in_=idx_xb[:])
            nc.sync.dma_start(out=irec_sb[:], in_=idx_rec[:])
            nc.sync.dma_start(out=snorm_sb[:], in_=snorm[:])
            gamma_sb = beta_sb = None
            if use_gamma:
                gamma_sb = cpool.tile([P, P], f32, tag="gam")
                nc.sync.dma_start(out=gamma_sb[:], in_=gamma_b[:])
            if use_beta:
                beta_sb = cpool.tile([P, P], f32, tag="bet")
                nc.sync.dma_start(out=beta_sb[:], in_=beta_b[:])

            # ---- node tables: xa (core slice), xb (all nodes)
            def table_mm(x_src, col0, w_sb, dram, row0):
                xt = xpool.tile([P, P], bf16, tag="xt")
                nc.sync.dma_start(out=xt[:], in_=x_src[:, col0:col0 + P])
                ps = psA.tile([P, P], f32, tag="psm")
                nc.tensor.matmul(out=ps[:], lhsT=xt[:], rhs=w_sb[:],
                                 start=True, stop=True)
                t = tpool.tile([P, P], f32, tag="tabout")
                nc.vector.tensor_copy(out=t[:], in_=ps[:])
                nc.sync.dma_start(out=dram[row0:row0 + P, :], in_=t[:])

            for j in range(n_xt // P):
                table_mm(x_t, j * P, w1bT_sb, xb_dram, j * P)

            # ---- phase 1 + 1.5, per block
            inv_d = 1.0 / P
            for b in (range(p.nblk) if stage not in ("tables",) else []):
                base_t = b * p.t_blk          # first tile of block
                base_e = base_t * P

                h_sb = bpool.tile([P, p.t_blk * P], bf16, tag="hblk")
                nc.sync.dma_start(out=h_sb[:],
                                  in_=h_t[:, base_e: base_e + p.t_blk * P])

                # xa for this block's 128 dst nodes: computed on PE, kept in
                # SBUF; per-edge selection happens via the transposed one-hot.
                xt_b = xpool.tile([P, P], bf16, tag="xtb")
                nc.sync.dma_start(out=xt_b[:], in_=x_tl[:, b * P:(b + 1) * P])
                ps_xa = psY.tile([P, P], f32, tag="psy")
                nc.tensor.matmul(out=ps_xa[:], lhsT=xt_b[:], rhs=w1aT_sb[:],
                                 start=True, stop=True)
                xa_sb = bpool.tile([P, P], bf16, tag="xasb")
                nc.scalar.copy(out=xa_sb[:], in_=ps_xa[:])

                xb_g = bpool.tile([P, p.t_blk, P], f32, tag="xbg")
                gather_tiles(xb_g, xb_dram[:lo_rows, :], ixb_sb, base_e, p.tl, P)
                if p.th > 0 and hi_rows > 0:
                    gather_tiles(xb_g, xb_dram[lo_rows:, :], ixb_sb,
                                 base_e + p.tl * P, p.th, P, tile_off=p.tl)

                if stage == "p1load":
                    continue
                ps_seg = psSeg.tile([P, P], f32, tag="seg")
                for tt in range(p.t_blk):
                    oh = epool.tile([P, P], bf16, tag="oh")
                    t_glob = base_t + tt
                    nc.vector.tensor_scalar(
                        out=oh[:], in0=iota_sb[:],
                        scalar1=dstrel_sb[:, t_glob:t_glob + 1], scalar2=None,
                        op0=mybir.AluOpType.is_equal)
                    ps_t = psT.tile([P, P], bf16, tag="pst")
                    nc.tensor.transpose(ps_t[:], oh[:], ident_sb[:])
                    ohT = epool.tile([P, P], bf16, tag="ohT")
                    nc.scalar.copy(out=ohT[:], in_=ps_t[:])
                    ps_m = psA.tile([P, P], f32, tag="psm")
                    nc.tensor.matmul(out=ps_m[:],
                                     lhsT=h_sb[:, tt * P:(tt + 1) * P],
                                     rhs=w1cT_sb[:], start=True, stop=False)
                    nc.tensor.matmul(out=ps_m[:], lhsT=ohT[:], rhs=xa_sb[:],
                                     start=False, stop=True)
                    tsum2 = epool.tile([P, P], f32, tag="tsum2")
                    nc.vector.tensor_tensor(out=tsum2[:], in0=xb_g[:, tt, :],
                                            in1=ps_m[:], op=mybir.AluOpType.add)
                    me = epool.tile([P, P], bf16, tag="me")
                    nc.scalar.activation(
                        out=me[:], in_=tsum2[:],
                        func=mybir.ActivationFunctionType.Relu)
                    nc.tensor.matmul(out=ps_seg[:], lhsT=me[:], rhs=oh[:],
                                     start=(tt == 0), stop=(tt == p.t_blk - 1))

                if stage == "p1mm":
                    continue
                # phase 1.5: y, stats, record
                mnT = npool.tile([P, P], bf16, tag="mnT")
                nc.vector.tensor_copy(out=mnT[:], in_=ps_seg[:])
                ps_y = psY.tile([P, P], f32, tag="psy")
                nc.tensor.matmul(out=ps_y[:], lhsT=mnT[:], rhs=w2T_sb[:],
                                 start=True, stop=True)

                rec = npool.tile([P, REC_W], f32, tag="rec")
                nc.vector.memset(rec[:], 0.0)
                if stage == "p1y":
                    nc.vector.tensor_copy(out=rec[:, 0:P], in_=ps_y[:])
                    nc.sync.dma_start(out=rec_local[b * P:(b + 1) * P, :],
                                      in_=rec[:])
                    continue
                mu = npool.tile([P, 1], f32, tag="mu")
                nc.vector.tensor_reduce(out=mu[:], in_=ps_y[:],
                                        axis=mybir.AxisListType.X,
                                        op=mybir.AluOpType.add)
                nc.vector.tensor_scalar_mul(mu[:], mu[:], inv_d)
                nc.vector.tensor_scalar(
                    out=rec[:, 0:P], in0=ps_y[:], scalar1=mu[:], scalar2=None,
                    op0=mybir.AluOpType.subtract)
                if stage == "p1stats":
                    nc.sync.dma_start(out=rec_local[b * P:(b + 1) * P, :],
                                      in_=rec[:])
                    continue
                sq = npool.tile([P, P], f32, tag="sq")
                nc.vector.tensor_tensor(out=sq[:], in0=rec[:, 0:P],
                                        in1=rec[:, 0:P],
                                        op=mybir.AluOpType.mult)
                vsum = npool.tile([P, 1], f32, tag="vsum")
                nc.vector.tensor_reduce(out=vsum[:], in_=sq[:],
                                        axis=mybir.AxisListType.X,
                                        op=mybir.AluOpType.add)
                nc.vector.tensor_scalar_mul(rec[:, P:P + 1], vsum[:], inv_d)
                if use_gamma:
                    nc.vector.tensor_tensor(out=rec[:, 0:P], in0=rec[:, 0:P],
                                            in1=gamma_sb[:],
                                            op=mybir.AluOpType.mult)
                nc.sync.dma_start(out=rec_local[b * P:(b + 1) * P, :],
                                  in_=rec[:])

            # ---- AllGather records
            if stage in ("ag", "full"):
                nc.gpsimd.collective_compute(
                "AllGather", mybir.AluOpType.bypass,
                    replica_groups=[list(range(p.nc))],
                    ins=[rec_local[:]], outs=[rec_full[:]])

            # ---- phase 2
            ct = p.p2_chunk_tiles
            n_chunks = p.t2 // ct if stage == "full" else 0
            rec_lo_rows = min(p.lo_limit, p.n_pad)
            for ch in range(n_chunks):
                t0 = ch * ct
                e0 = t0 * P
                is_hi = t0 >= p.lo2
                rec_g = p2pool.tile([P, ct, REC_W], f32, tag="recg")
                src_ap = rec_full[rec_lo_rows:, :] if is_hi else \
                    rec_full[:rec_lo_rows, :]
                gather_tiles(rec_g, src_ap, irec_sb, e0, ct, REC_W)

                # batched per-chunk LN scale: a_e = s*rsqrt(s^2*var + eps)
                sn_ap = snorm_sb[:, t0:t0 + ct]
                s2 = p2pool.tile([P, ct], f32, tag="s2")
                nc.vector.tensor_tensor(out=s2[:], in0=sn_ap, in1=sn_ap,
                                        op=mybir.AluOpType.mult)
                q2 = p2pool.tile([P, ct], f32, tag="q2")
                nc.vector.tensor_tensor(out=q2[:], in0=rec_g[:, :, P:P + 1],
                                        in1=s2[:], op=mybir.AluOpType.mult)
                q = p2pool.tile([P, ct], f32, tag="q")
                nc.scalar.activation(out=q[:], in_=q2[:],
                                     func=mybir.ActivationFunctionType.Sqrt,
                                     bias=eps_sb[:])
                rq = p2pool.tile([P, ct], f32, tag="rq")
                nc.vector.reciprocal(out=rq[:], in_=q[:])
                a = p2pool.tile([P, ct], f32, tag="a")
                nc.vector.tensor_tensor(out=a[:], in0=rq[:], in1=sn_ap,
                                        op=mybir.AluOpType.mult)

                out_sb = p2pool.tile([P, ct, P], f32, tag="outsb")
                for tt in range(ct):
                    if use_beta:
                        t1 = p2pool.tile([P, P], f32, tag="t1")
                        nc.vector.tensor_scalar(
                            out=t1[:], in0=rec_g[:, tt, 0:P],
                            scalar1=a[:, tt:tt + 1],
                            scalar2=None, op0=mybir.AluOpType.mult)
                        nc.vector.tensor_tensor(out=t1[:], in0=t1[:],
                                                in1=beta_sb[:],
                                                op=mybir.AluOpType.add)
                        nc.scalar.activation(
                            out=out_sb[:, tt, :], in_=t1[:],
                            func=mybir.ActivationFunctionType.Relu)
                    else:
                        nc.scalar.activation(
                            out=out_sb[:, tt, :], in_=rec_g[:, tt, 0:P],
                            func=mybir.ActivationFunctionType.Relu,
                            scale=a[:, tt:tt + 1])

                out_view = out[e0: e0 + ct * P, :].rearrange(
                    "(t p) d -> p t d", p=P)
                nc.sync.dma_start(out=out_view, in_=out_sb[:])

    nc.finalize()
    return nc


# ----------------------------------------------------------------------------
# driver
# ----------------------------------------------------------------------------


def _prep_inputs(p: Plan, x, h, snorm_n, W1, W2, ln_gamma, ln_beta):
    D = P
    use_gamma = not np.allclose(ln_gamma, 1.0)
    use_beta = not np.allclose(ln_beta, 0.0)

    x_t_full = np.zeros((D, p.n_pad), dtype=BF16)
    # x.T laid out per-core-slice: table row (c*npc_pad + i) = node c*npc + i
    xt = np.asarray(x).T.astype(BF16)
    for c in range(p.nc):
        x_t_full[:, c * p.npc_pad: c * p.npc_pad + p.npc] = \
            xt[:, c * p.npc: (c + 1) * p.npc]

    common = {
        "x_t": x_t_full,
        "w1aT": np.ascontiguousarray(W1[:, :D].T).astype(BF16),
        "w1bT": np.ascontiguousarray(W1[:, D:2 * D].T).astype(BF16),
        "w1cT": np.ascontiguousarray(W1[:, 2 * D:3 * D].T).astype(BF16),
        "w2T": np.ascontiguousarray(W2.T).astype(BF16),
        "ident": np.eye(P, dtype=np.float32).astype(BF16),
        "iota": np.tile(np.arange(P, dtype=np.float32), (P, 1)),
    }
    if use_gamma:
        common["gamma_b"] = np.tile(np.asarray(ln_gamma, np.float32), (P, 1))
    if use_beta:
        common["beta_b"] = np.tile(np.asarray(ln_beta, np.float32), (P, 1))

    in_maps, slots2_all = [], []
    for c in range(p.nc):
        m, slots2 = p.core_inputs(c, x, h, snorm_n, W1, W2)
        m.update(common)
        m["x_tl"] = np.ascontiguousarray(
            x_t_full[:, c * p.npc_pad: (c + 1) * p.npc_pad])
        in_maps.append(m)
        slots2_all.append(slots2)
    return in_maps, slots2_all, use_gamma, use_beta


def run(x, h, snorm_n, W1, W2, ln_gamma, ln_beta, src, dst, n_cores=8,
        lo_limit=32768, trace=False, stage="full"):
    n_nodes, n_edges = x.shape[0], h.shape[0]
    p = Plan(n_nodes, n_edges, src, dst, nc=n_cores, lo_limit=lo_limit)
    in_maps, slots2_all, use_gamma, use_beta = _prep_inputs(
        p, x, h, snorm_n, W1, W2, ln_gamma, ln_beta)
    nc = build_program(p, use_gamma, use_beta, stage=stage)
    res = run_bass_kernel_spmd(nc, in_maps, core_ids=list(range(n_cores)),
                               trace=trace)
    out = np.empty((n_edges, P), dtype=np.float32)
    for c in range(n_cores):
        o = res.results[c]["out"]
        s = slots2_all[c]
        real = s >= 0
        out[s[real]] = o[real]
    return out, res


def kernel(x, h, snorm_n, snorm_e, W1, W2, ln_gamma, ln_beta, src, dst):
    out, _ = run(np.asarray(x), np.asarray(h), np.asarray(snorm_n),
                 np.asarray(W1), np.asarray(W2), np.asarray(ln_gamma),
                 np.asarray(ln_beta), np.asarray(src), np.asarray(dst))
    return out

